# revision 2
# baseline (speedup 1.0000x reference)
"""ConvLSTM classifier kernel v3: v2 + batched DMAs.

  - ONE xh tile per parity [128, nbg*1024] (bg-slices), so the per-timestep
    x load is ONE contiguous HWDGE DMA (16 x 2KB runs per partition) instead
    of 16, and the xbar transpose runs per 4-bg group (4 calls/t instead of
    16) -> ~20x fewer DMA instructions than v2.
  - everything else identical to v2.
"""

import numpy as np

import concourse.bass as bass
import concourse.bacc as bacc
import concourse.tile as tile
import concourse.mybir as mybir
from concourse import bass_utils

dt = mybir.dt
ALU = mybir.AluOpType
ACT = mybir.ActivationFunctionType

TIME = 16
BATCH = 16384
C = 5
L = 64
NCORES = 8
BL = BATCH // NCORES          # 2048 per core
NW = 8                        # l-windows per row (l_seg = 8)
WJ = 12                       # taps per (window, channel): 8 + 4 halo
ROW = NW * 128                # 1024 cols per b-major row
TG = 4                        # bgs per transpose group

X_OFF = 0
H_OFF = 64
BIAS_COL = 60


def make_wband(w_ih, w_hh, b_ih, b_hh):
    refbase = (0, 5, 15, 10)  # i, f, o, g -> reference channel offsets
    wb = np.zeros((128, 160), np.float32)
    for row0, wmat in ((X_OFF, w_ih), (H_OFF, w_hh)):
        for c in range(C):
            for j in range(WJ):
                r = row0 + c * WJ + j
                for G in range(4):
                    scale = 2.0 if G == 3 else 1.0
                    for ch in range(C):
                        for lam in range(NW):
                            k = j - lam
                            if 0 <= k < 5:
                                wb[r, G * 40 + ch * 8 + lam] = (
                                    scale * wmat[refbase[G] + ch, c, k]
                                )
    bias = (np.asarray(b_ih) + np.asarray(b_hh)).astype(np.float32)
    for G in range(4):
        scale = 2.0 if G == 3 else 1.0
        for ch in range(C):
            for lam in range(NW):
                wb[BIAS_COL, G * 40 + ch * 8 + lam] = scale * bias[refbase[G] + ch]
    return wb.astype(np.float16)


def _ap(base, off, dims):
    return bass.AP(
        tensor=base.tensor,
        offset=base.offset + off,
        ap=[list(base.ap[0])] + [list(d) for d in dims],
    )


def build_body(tc, out_dram, xs, wband_d, fcw5_d, consts_d, T, nbg, reps=1):
    nc = tc.nc
    f16, f32 = dt.float16, dt.float32

    from contextlib import ExitStack
    es = ExitStack()
    pers = es.enter_context(tc.tile_pool(name="pers", bufs=1))
    psum_pool = es.enter_context(tc.tile_pool(name="psum", bufs=2, space="PSUM"))
    ifog_pool = es.enter_context(tc.tile_pool(name="ifog", bufs=6))
    small = es.enter_context(tc.tile_pool(name="small", bufs=6))
    xht_pool = es.enter_context(tc.tile_pool(name="xht", bufs=3))
    fin_pool = es.enter_context(tc.tile_pool(name="fin", bufs=2))

    wband = pers.tile([128, 160], f16, tag="wband")
    nc.sync.dma_start(out=wband, in_=wband_d)
    fcw5 = pers.tile([128, C * L], f16, tag="fcw5")
    nc.gpsimd.dma_start(
        out=fcw5,
        in_=bass.AP(tensor=fcw5_d.tensor, offset=fcw5_d.offset,
                    ap=[[0, 128], [1, C * L]]),
    )
    consts = pers.tile([128, 2], f32, tag="consts")
    nc.gpsimd.dma_start(
        out=consts,
        in_=bass.AP(tensor=consts_d.tensor, offset=consts_d.offset,
                    ap=[[0, 128], [1, 2]]),
    )
    fcbneg = consts[:, 0:1]
    negq = consts[:, 1:2]

    # one b-major tile per parity; bg bg occupies cols [bg*ROW, (bg+1)*ROW)
    xh = [pers.tile([128, nbg * ROW], f16, tag=f"xh{pp}", name=f"xh{pp}")
          for pp in range(2)]

    npair = (nbg + 1) // 2
    cbuf = [[pers.tile([128, 640], f16, tag=f"c{pp}_{pr}", name=f"c{pp}_{pr}")
             for pr in range(npair)] for pp in range(2)]
    for pr in range(npair):
        nc.vector.memset(cbuf[0][pr], 0.0)
    tpair = [pers.tile([128, 640], f16, tag=f"t{pr}", name=f"t{pr}")
             for pr in range(npair)]

    res_tile = pers.tile([128, nbg], f32, tag="res")

    def load_x(t):
        # xs[t] is [nbg*128, ROW] in DRAM; SBUF partitions = b%128, free
        # dims (bg, col): one 3D-AP DMA, 2KB contiguous runs.
        nc.sync.dma_start(
            out=xh[t % 2][:],
            in_=_ap(xs[t, 0:128, :], 0, [[128 * ROW, nbg], [1, ROW]]),
        )

    o_slices = {}
    for rep in range(reps):
      load_x(0)
      for t in range(T):
        if t + 1 < T:
            load_x(t + 1)
        c_old, c_new = cbuf[t % 2], cbuf[(t + 1) % 2]
        xh_t = xh[t % 2][:]
        xh_n = xh[(t + 1) % 2][:]
        for g in range(nbg // TG):
            xht = xht_pool.tile([128, TG * NW, 128], f16, tag="xht")
            nc.sync.dma_start(
                out=xht[:],
                in_=_ap(xh_t, g * TG * ROW, [[1, TG * ROW]]),
                transpose=True,
            )
            for bgl in range(TG):
              bg = g * TG + bgl
              slot = psum_pool.tile([128, 4 * 512], f32, tag="gates")
              for w in range(NW):
                col = (w // 2) * 512 + (w % 2) * 160
                nc.tensor.matmul(
                    slot[:, col : col + 160],
                    lhsT=xht[:, bgl * NW + w, :],
                    rhs=wband[:],
                    start=True,
                    stop=True,
                )

              ifog = ifog_pool.tile([128, NW * 160], f16, tag="ifog")
              nc.scalar.activation(
                  out=_ap(ifog[:], 0, [[320, 4], [160, 2], [1, 160]]),
                  in_=_ap(slot[:], 0, [[512, 4], [160, 2], [1, 160]]),
                  func=ACT.Sigmoid,
              )

              ifog_f = ifog[:]
              sl_i = _ap(ifog_f, 0, [[160, NW], [1, 40]])
              sl_f = _ap(ifog_f, 40, [[160, NW], [1, 40]])
              sl_g = _ap(ifog_f, 120, [[160, NW], [1, 40]])
              o_slices[bg] = _ap(ifog_f, 80, [[160, NW], [8, C], [1, 8]])

              v = small.tile([128, 320], f16, tag="v")
              nc.vector.tensor_tensor(out=v, in0=sl_i, in1=sl_g, op=ALU.mult)
              u = small.tile([128, 320], f16, tag="u")
              nc.vector.scalar_tensor_tensor(
                  out=u, in0=v[:], scalar=2.0, in1=sl_i,
                  op0=ALU.mult, op1=ALU.subtract,
              )
              co = c_old[bg // 2][:, (bg % 2) * 320 : (bg % 2 + 1) * 320]
              cn = c_new[bg // 2][:, (bg % 2) * 320 : (bg % 2 + 1) * 320]
              fc = small.tile([128, 320], f16, tag="fc")
              nc.vector.tensor_tensor(out=fc, in0=sl_f, in1=co, op=ALU.mult)
              nc.vector.tensor_tensor(out=cn, in0=fc[:], in1=u[:], op=ALU.add)

              if bg % 2 == 1 or bg == nbg - 1:
                blo = bg - 1 if bg % 2 == 1 else bg
                pr = bg // 2
                w_hi = (bg % 2 + 1) * 320
                nc.scalar.activation(
                    out=tpair[pr][:, 0:w_hi], in_=c_new[pr][:, 0:w_hi],
                    func=ACT.Tanh
                )
                for b2 in range(blo, bg + 1):
                    tsl = _ap(tpair[pr][:, (b2 % 2) * 320 : (b2 % 2 + 1) * 320],
                              0, [[40, NW], [8, C], [1, 8]])
                    base = b2 * ROW
                    hdst = _ap(xh_n, base + H_OFF + 2,
                               [[128, NW], [WJ, C], [1, 8]])
                    nc.vector.tensor_tensor(
                        out=hdst, in0=o_slices[b2], in1=tsl, op=ALU.mult
                    )
                    nc.vector.tensor_copy(
                        out=_ap(xh_n, base + 128 + H_OFF,
                                [[128, NW - 1], [WJ, C], [1, 2]]),
                        in_=_ap(xh_n, base + H_OFF + 8,
                                [[128, NW - 1], [WJ, C], [1, 2]]),
                    )
                    nc.vector.tensor_copy(
                        out=_ap(xh_n, base + H_OFF + 10,
                                [[128, NW - 1], [WJ, C], [1, 2]]),
                        in_=_ap(xh_n, base + 128 + H_OFF + 2,
                                [[128, NW - 1], [WJ, C], [1, 2]]),
                    )

    # --- final FC / combine ---
    for bg in range(nbg):
        hview = _ap(xh[T % 2][:], bg * ROW + H_OFF + 2,
                    [[128, NW], [WJ, C], [1, 8]])
        fview = _ap(fcw5[:], 0, [[8, NW], [L, C], [1, 8]])
        tmp5 = fin_pool.tile([128, C * L], f32, tag="tmp5")
        tview = _ap(tmp5[:], 0, [[8, NW], [L, C], [1, 8]])
        nc.vector.tensor_tensor(out=tview, in0=hview, in1=fview, op=ALU.mult)
        nraw = fin_pool.tile([128, C], f32, tag="nraw")
        nc.vector.tensor_reduce(
            out=nraw,
            in_=tmp5[:].rearrange("p (c l) -> p c l", l=L),
            axis=mybir.AxisListType.X,
            op=ALU.add,
        )
        pbar = fin_pool.tile([128, C], f32, tag="pbar")
        nc.scalar.activation(
            out=pbar, in_=nraw[:], func=ACT.Sigmoid, bias=fcbneg, scale=1.0
        )
        q2 = fin_pool.tile([128, 2], f32, tag="q2")
        nc.vector.tensor_tensor(out=q2, in0=pbar[:, 0:2], in1=pbar[:, 2:4],
                                op=ALU.mult)
        prod = fin_pool.tile([128, 1], f32, tag="prod")
        nc.vector.tensor_tensor(out=prod, in0=q2[:, 0:1], in1=q2[:, 1:2],
                                op=ALU.mult)
        nc.vector.tensor_tensor(out=prod, in0=prod[:], in1=pbar[:, 4:5],
                                op=ALU.mult)
        nc.scalar.activation(
            out=res_tile[:, bg : bg + 1], in_=prod[:], func=ACT.Identity,
            bias=1.0, scale=negq,
        )
    nc.sync.dma_start(out=out_dram, in_=res_tile[:])
    es.close()


def window_x(x):
    from numpy.lib.stride_tricks import sliding_window_view
    T_, B_ = x.shape[0], x.shape[1]
    xp = np.pad(x, ((0, 0), (0, 0), (0, 0), (2, 2)))
    win = sliding_window_view(xp, WJ, axis=3)[:, :, :, ::NW, :]  # T,B,C,NW,WJ
    out = np.zeros((T_, B_, NW, 128), np.float16)
    out[:, :, :, : C * WJ] = (
        win.transpose(0, 1, 3, 2, 4).reshape(T_, B_, NW, C * WJ)
    )
    out[:, :, :, BIAS_COL] = 1.0
    return out.reshape(T_, B_, ROW)


def host_prep(w_ih, w_hh, b_ih, b_hh, fc_w, fc_b, baseline):
    wband = make_wband(np.asarray(w_ih), np.asarray(w_hh),
                       np.asarray(b_ih), np.asarray(b_hh))
    fcw = np.asarray(fc_w)[0].astype(np.float32)          # (64,)
    fcw5 = np.tile(-fcw, C)[None, :].astype(np.float16)    # (1, 320)
    base = float(np.asarray(baseline)[0])
    sig_base = 1.0 / (1.0 + np.exp(-base))
    consts = np.array([[-float(np.asarray(fc_b)[0]), -(1.0 - sig_base)]],
                      np.float32)
    return wband, fcw5, consts


def build_program(T, nbg, reps=1):
    nc = bacc.Bacc("TRN2", target_bir_lowering=False, debug=False, num_devices=1)
    xs = nc.dram_tensor("xs", [T, nbg * 128, ROW], dt.float16,
                        kind="ExternalInput").ap()
    wband_d = nc.dram_tensor("wband", [128, 160], dt.float16,
                             kind="ExternalInput").ap()
    fcw5_d = nc.dram_tensor("fcw5", [1, C * L], dt.float16,
                            kind="ExternalInput").ap()
    consts_d = nc.dram_tensor("consts", [1, 2], dt.float32,
                              kind="ExternalInput").ap()
    out_d = nc.dram_tensor("out", [128, nbg], dt.float32,
                           kind="ExternalOutput").ap()
    with tile.TileContext(nc) as tc:
        build_body(tc, out_d, xs, wband_d, fcw5_d, consts_d, T, nbg, reps=reps)
    nc.compile()
    return nc


_PROG_CACHE = {}


def kernel(x, w_ih, w_hh, b_ih, b_hh, fc_w, fc_b, baseline):
    x = np.asarray(x)
    T, B = x.shape[0], x.shape[1]
    nbg = (B // NCORES) // 128
    key = (T, nbg)
    if key not in _PROG_CACHE:
        _PROG_CACHE[key] = build_program(T, nbg)
    nc = _PROG_CACHE[key]

    wband, fcw5, consts = host_prep(w_ih, w_hh, b_ih, b_hh, fc_w, fc_b, baseline)
    xw = window_x(x)
    bl = B // NCORES
    in_maps = []
    for core in range(NCORES):
        in_maps.append({
            "xs": np.ascontiguousarray(xw[:, core * bl : (core + 1) * bl]),
            "wband": wband,
            "fcw5": fcw5,
            "consts": consts,
        })
    res = bass_utils.run_bass_kernel_spmd(nc, in_maps, core_ids=list(range(NCORES)))
    out = np.concatenate([r["out"].T.reshape(-1) for r in res.results])
    return out.astype(np.float32)
